# revision 5
# baseline (speedup 1.0000x reference)
"""WENO5 2D advection tendency kernel v3 for 8 Trainium2 NeuronCores.

Changes vs v2:
  * all inputs host-converted to bf16 (the device cast to bf16 happened
    anyway) -> no Act cast copies, half the DMA bytes
  * upwind velocity split (relu(vel)*-1/D, min(vel,0)*-1/D) precomputed on
    host for both directions -> no device TSP splits
  * y-pass fields transposed AND tile-interleaved on host:
    hty[g][p, k, :] = hT[(g*G+k)*128 + p, :] so one contiguous DMA yields
    [128, G, rpc+6] y-major tiles directly -> no PE input transposes, no
    PSUM evacuation copies
  * engine rebalance: 9 TT op classes (mult/add only) moved to the idle
    GPSIMD/Pool engine, prescale TS ops on Act, rest on DVE
"""

import numpy as np

import concourse.bass as bass
import concourse.bacc as bacc
import concourse.mybir as mybir
from concourse.tile import TileContext

F32 = mybir.dt.float32
BF16 = mybir.dt.bfloat16
AF = mybir.ActivationFunctionType
MUL = mybir.AluOpType.mult
ADD = mybir.AluOpType.add
SUB = mybir.AluOpType.subtract
MAX = mybir.AluOpType.max
MIN = mybir.AluOpType.min

DX = 1000.0
DY = 1000.0
EPS = 1e-8
K1 = 13.0 / 12.0
K2 = 0.25
import math
SK2 = math.sqrt(K2 / K1)
EPSP = EPS / K1

NY, NX = 2048, 4096
N_CORES = 8
RPC = NY // N_CORES   # rows per core
XC = 1024             # x-chunk width
G = 4                 # x-tiles per y-group
NG = NX // (128 * G)  # y-groups


class _Arr:
    """A tile plus the global index its local free element 0 maps to."""

    def __init__(self, tile, base):
        self.tile = tile
        self.base = base

    def full(self):
        return self.tile[:]

    def win(self, g0, w):
        o = g0 - self.base
        ap = self.tile
        assert o >= 0, (self.base, g0)
        if len(ap.shape) == 2:
            return ap[:, o:o + w]
        return ap[:, :, o:o + w]


class _Slots:
    """Manual lifetime manager: a small set of rotating pool tags."""

    def __init__(self, pool, shape3, nslots=14, nslots_f32=3, prefix="w",
                 bufs=None, dtype=BF16):
        self.pool = pool
        self.shape3 = shape3
        self.bufs = bufs
        self.dtype = dtype
        self.free_tags = [f"{prefix}{i}" for i in range(nslots)]
        self.free_tags_f32 = [f"{prefix}f{i}" for i in range(nslots_f32)]
        self.tags = {}

    def alloc(self, name, w, base, dtype=None):
        f32 = dtype is F32
        tag = (self.free_tags_f32 if f32 else self.free_tags).pop()
        s = list(self.shape3)
        s[-1] = w
        t = self.pool.tile(s, dtype or self.dtype, tag=tag,
                           name=f"{tag}_{name}", bufs=self.bufs)
        self.tags[name] = (tag, f32)
        return _Arr(t, base)

    def alloc_f32(self, name, w, base):
        return self.alloc(name, w, base, dtype=F32)

    def free(self, *names):
        for n in names:
            tag, f32 = self.tags.pop(n)
            (self.free_tags_f32 if f32 else self.free_tags).append(tag)


# Engine assignment: 'v' = DVE, 'p' = Pool/GPSIMD (TT add/mult ONLY — other
# opcodes fail neuronxcc codegen on Pool), 'a' = Act.
DEFAULT_ASG = {
    "dq": "v", "dqa": "a", "dqb": "a", "dq6": "a", "dq3": "a",
    "d2": "v", "d2sq": "a",
    "l2": "v", "l3": "v", "l4": "v",
    "x2": "a", "x3": "a", "x4": "a",
    "y1": "v", "y2": "p", "y3": "p",
    "b1": "a", "b2": "a", "b3": "a",
    "wd": "v",
    "p12": "p", "p13": "v", "p23": "p",
    "p23s": "v", "p12s": "v", "p13s": "v",
    "u1": "v", "denl": "v", "u2": "v", "denr": "v",
    "pa": "p", "pb": "p", "pa15": "v", "pb15": "v",
    "numl": "v", "numr": "v",
    "gl": "v", "gr": "v",
    "tf": "p", "tg": "p",
    "al": "v", "ql": "v", "ar": "v", "qr": "v",
    "t1": "v", "t2": "v", "fe": "v",
}


def _flux_v2(nc, sl, H, UP, UM, F0, WF, NF, asg):
    """Decomposed WENO5 upwind flux, pre-scaled by the UP/UM factors.

    Generator: yields after each emitted op (for software pipelining);
    its StopIteration value is the flux _Arr over faces [F0, F0+NF).
    """
    E = {"v": nc.vector, "p": nc.gpsimd, "a": nc.scalar}
    A = nc.scalar

    def tt(name, out, in0, in1, op):
        E[asg[name]].tensor_tensor(out, in0, in1, op)

    def ts(name, out, in0, s):  # out = in0 * s
        if asg[name] == "a":
            A.activation(out, in0, AF.Copy, scale=float(s))
        else:
            E[asg[name]].tensor_scalar(out, in0, float(s), None, MUL)

    def sq(name, out, in0):  # out = in0^2
        if asg[name] == "a":
            A.activation(out, in0, AF.Square)
        else:
            E[asg[name]].tensor_tensor(out, in0, in0, MUL)

    ncell = WF + 3
    dq = sl.alloc("dq", ncell, F0 - 2)
    tt("dq", dq.full(), H.win(F0 - 1, ncell), H.win(F0 - 2, ncell), SUB)
    yield

    # pre-scaled dq copies (shared by several linear combos)
    dqa = sl.alloc("dqa", ncell, F0 - 2)   # 3*sk*dq
    ts("dqa", dqa.full(), dq.full(), 3.0 * SK2)
    yield
    dqb = sl.alloc("dqb", ncell, F0 - 2)   # sk*dq
    ts("dqb", dqb.full(), dq.full(), SK2)
    yield
    dq6 = sl.alloc("dq6", ncell, F0 - 2)   # dq/6
    ts("dq6", dq6.full(), dq.full(), 1.0 / 6.0)
    yield
    dq3 = sl.alloc("dq3", ncell, F0 - 2)   # dq/3
    ts("dq3", dq3.full(), dq.full(), 1.0 / 3.0)
    yield

    nd2 = WF + 2
    d2 = sl.alloc("d2", nd2, F0 - 1)
    tt("d2", d2.full(), dq.win(F0 - 1, nd2), dq.win(F0 - 2, nd2), SUB)
    sl.free("dq")
    yield

    d2sq = sl.alloc("d2sq", nd2, F0 - 1)  # d2^2 (K1 folded out)
    sq("d2sq", d2sq.full(), d2.full())
    yield

    # scaled linear beta combos: l = sk * (stencil); X = l^2
    l2 = sl.alloc("l2", WF, F0)
    tt("l2", l2.full(), dqa.win(F0 - 1, WF), dqb.win(F0 - 2, WF), SUB)
    yield
    l3 = sl.alloc("l3", WF, F0)
    tt("l3", l3.full(), dqb.win(F0 - 1, WF), dqb.win(F0, WF), ADD)
    yield
    l4 = sl.alloc("l4", WF, F0)
    tt("l4", l4.full(), dqa.win(F0, WF), dqb.win(F0 + 1, WF), SUB)
    sl.free("dqa", "dqb")
    yield

    X2 = sl.alloc("x2", WF, F0)
    sq("x2", X2.full(), l2.full())
    yield
    X3 = sl.alloc("x3", WF, F0)
    sq("x3", X3.full(), l3.full())
    yield
    X4 = sl.alloc("x4", WF, F0)
    sq("x4", X4.full(), l4.full())
    sl.free("l2", "l3", "l4")
    yield

    nwd = WF + 1
    Wd = sl.alloc("wd", nwd, F0 - 1)  # Wd[f] = d2[f] - d2[f+1]
    tt("wd", Wd.full(), d2.win(F0 - 1, nwd), d2.win(F0, nwd), SUB)
    sl.free("d2")
    yield

    # Y = X + d2sq(shift);  B = (Y + eps')^2
    Y1 = sl.alloc("y1", WF, F0)
    tt("y1", Y1.full(), X2.full(), d2sq.win(F0 - 1, WF), ADD)
    yield
    Y2 = sl.alloc("y2", WF, F0)
    tt("y2", Y2.full(), X3.full(), d2sq.win(F0, WF), ADD)
    yield
    Y3 = sl.alloc("y3", WF, F0)
    tt("y3", Y3.full(), X4.full(), d2sq.win(F0 + 1, WF), ADD)
    sl.free("x2", "x3", "x4", "d2sq")
    yield

    def beta(name, y_arr):
        b = sl.alloc(name, WF, F0)
        if asg[name] == "a":
            A.activation(b.full(), y_arr.full(), AF.Square, bias=EPSP)
        else:
            ye = sl.alloc(name + "e", WF, F0)
            E[asg[name]].tensor_scalar(ye.full(), y_arr.full(), EPSP, None,
                                       ADD)
            E[asg[name]].tensor_tensor(b.full(), ye.full(), ye.full(), MUL)
            sl.free(name + "e")
        return b

    B1 = beta("b1", Y1)
    yield
    B2 = beta("b2", Y2)
    yield
    B3 = beta("b3", Y3)
    sl.free("y1", "y2", "y3")
    yield

    p12 = sl.alloc("p12", WF, F0)
    tt("p12", p12.full(), B1.full(), B2.full(), MUL)
    yield
    p13 = sl.alloc("p13", WF, F0)
    tt("p13", p13.full(), B1.full(), B3.full(), MUL)
    yield
    p23 = sl.alloc("p23", WF, F0)
    tt("p23", p23.full(), B2.full(), B3.full(), MUL)
    sl.free("b1", "b2", "b3")
    yield

    # den (x30): denL3 = 3 p23 + 18 p13 + 9 p12; denR3 mirrored.
    p13_18 = sl.alloc("p13_18", WF, F0)
    ts("p13s", p13_18.full(), p13.full(), 18.0)
    sl.free("p13")
    yield
    p12_9 = sl.alloc("p12_9", WF, F0)
    ts("p12s", p12_9.full(), p12.full(), 9.0)
    yield
    a1 = sl.alloc("a1", WF, F0)
    tt("u1", a1.full(), p13_18.full(), p12_9.full(), ADD)
    sl.free("p12_9")
    yield
    p23_9 = sl.alloc("p23_9", WF, F0)
    ts("p23s", p23_9.full(), p23.full(), 9.0)
    yield
    a2 = sl.alloc("a2", WF, F0)
    tt("u2", a2.full(), p13_18.full(), p23_9.full(), ADD)
    sl.free("p13_18", "p23_9")
    yield
    p23_3 = sl.alloc("p23_3", WF, F0)
    ts("p23s", p23_3.full(), p23.full(), 3.0)
    yield
    denL = sl.alloc_f32("denl", WF, F0)
    tt("denl", denL.full(), a1.full(), p23_3.full(), ADD)
    sl.free("a1", "p23_3")
    yield
    p12_3 = sl.alloc("p12_3", WF, F0)
    ts("p12s", p12_3.full(), p12.full(), 3.0)
    yield
    denR = sl.alloc_f32("denr", WF, F0)
    tt("denr", denR.full(), a2.full(), p12_3.full(), ADD)
    sl.free("a2", "p12_3")
    yield

    PA = sl.alloc("pa", WF, F0)
    tt("pa", PA.full(), p23.full(), Wd.win(F0 - 1, WF), MUL)
    yield
    PB = sl.alloc("pb", WF, F0)
    tt("pb", PB.full(), p12.full(), Wd.win(F0, WF), MUL)
    sl.free("p12", "p23", "wd")
    yield

    # numL = PA + 1.5 PB ; numR = PB + 1.5 PA  (gL = numL/denL3 = mL/3)
    pa15 = sl.alloc("pa15", WF, F0)
    ts("pa15", pa15.full(), PA.full(), 1.5)
    yield
    pb15 = sl.alloc("pb15", WF, F0)
    ts("pb15", pb15.full(), PB.full(), 1.5)
    yield
    numL = sl.alloc("numl", WF, F0)
    tt("numl", numL.full(), PA.full(), pb15.full(), ADD)
    yield
    numR = sl.alloc("numr", WF, F0)
    tt("numr", numR.full(), PB.full(), pa15.full(), ADD)
    sl.free("pa", "pb", "pa15", "pb15")
    yield

    rL = sl.alloc_f32("rl", WF, F0)
    nc.vector.reciprocal_approx_fast(out=rL.full(), in_=denL.full())
    sl.free("denl")
    yield
    rR = sl.alloc_f32("rr", WF, F0)
    nc.vector.reciprocal_approx_fast(out=rR.full(), in_=denR.full())
    sl.free("denr")
    yield
    gL = sl.alloc("gl", WF, F0)   # mL/3 = numL * (1/denL3)
    tt("gl", gL.full(), numL.full(), rL.full(), MUL)
    yield
    gR = sl.alloc("gr", WF, F0)   # mR/3
    tt("gr", gR.full(), numR.full(), rR.full(), MUL)
    sl.free("numl", "numr", "rl", "rr")
    yield

    # tF = dq[f-1]/6 + dq[f]/3 ; tG = dq[f+1]/6 + dq[f]/3
    tF = sl.alloc("tf", NF, F0)
    tt("tf", tF.full(), dq6.win(F0 - 1, NF), dq3.win(F0, NF), ADD)
    yield
    tG = sl.alloc("tg", NF, F0)
    tt("tg", tG.full(), dq6.win(F0 + 1, NF), dq3.win(F0, NF), ADD)
    sl.free("dq6", "dq3")
    yield

    # qL = gL + tF + h[f];  qR = h[f+1] - (gR[f+1] + tG)
    aL = sl.alloc("al", NF, F0)
    tt("al", aL.full(), gL.win(F0, NF), tF.full(), ADD)
    yield
    qL = sl.alloc("ql", NF, F0)
    tt("ql", qL.full(), aL.full(), H.win(F0, NF), ADD)
    yield
    aR = sl.alloc("ar", NF, F0)
    tt("ar", aR.full(), gR.win(F0 + 1, NF), tG.full(), ADD)
    yield
    qR = sl.alloc("qr", NF, F0)
    tt("qr", qR.full(), H.win(F0 + 1, NF), aR.full(), SUB)
    sl.free("gl", "gr", "tf", "tg", "al", "ar")
    yield

    t1 = sl.alloc("t1", NF, F0)
    tt("t1", t1.full(), UP.win(F0, NF), qL.full(), MUL)
    yield
    t2 = sl.alloc("t2", NF, F0)
    tt("t2", t2.full(), UM.win(F0, NF), qR.full(), MUL)
    sl.free("ql", "qr")
    yield

    fe = sl.alloc("fe", NF, F0)
    tt("fe", fe.full(), t1.full(), t2.full(), ADD)
    sl.free("t1", "t2")
    yield
    return fe


def _drive(gens, prime=22, depth=2):
    """Depth-N software pipeline: round-robin active generators, the
    first primed `prime` ops ahead so one call's tail overlaps the next
    call's head in every engine's in-order instruction stream."""
    from collections import deque
    it = iter(gens)
    act = deque()
    first = True
    while True:
        while len(act) < depth:
            g = next(it, None)
            if g is None:
                break
            act.append(g)
            if first:
                first = False
                for _ in range(prime):
                    try:
                        next(act[0])
                    except StopIteration:
                        act.popleft()
                        break
        if not act:
            return
        g = act[0]
        try:
            next(g)
            act.rotate(-1)
        except StopIteration:
            act.remove(g)


def build_program(rpc=RPC, nx=NX, xc=XC, g_seg=G, reps=1, hw_loop=False,
                  asg=None, work_bufs=1, nslots=14, prime=22,
                  depth=2, npfx=2):
    """SPMD Bass program computing one core's [rpc, nx] tendency block."""
    assert rpc % 128 == 0 and nx % xc == 0 and nx % (128 * g_seg) == 0
    asg = dict(DEFAULT_ASG, **(asg or {}))
    yb = rpc // 128
    nchunk = nx // xc
    ng = nx // (128 * g_seg)
    YW = rpc + 6
    VW = rpc + 1

    nc = bacc.Bacc("TRN2", target_bir_lowering=False, debug=False)
    # x-pass inputs row-block interleaved on host: [p, b, :] = row b*128+p
    hx_d = nc.dram_tensor("hx", [128, yb, nx + 6], BF16,
                          kind="ExternalInput")
    upx_d = nc.dram_tensor("upx", [128, yb, nx + 1], BF16,
                           kind="ExternalInput")
    umx_d = nc.dram_tensor("umx", [128, yb, nx + 1], BF16,
                           kind="ExternalInput")
    hty_d = nc.dram_tensor("hty", [ng * 128, g_seg, YW], BF16,
                           kind="ExternalInput")
    upy_d = nc.dram_tensor("upy", [ng * 128, g_seg, VW], BF16,
                           kind="ExternalInput")
    umy_d = nc.dram_tensor("umy", [ng * 128, g_seg, VW], BF16,
                           kind="ExternalInput")
    out_d = nc.dram_tensor("out", [rpc, nx], F32, kind="ExternalOutput")

    V = nc.vector
    A = nc.scalar

    # const AP for the Act Square(bias=EPSP) beta form
    _epst = nc.alloc_sbuf_tensor("const-epsp", [128, 1], F32)
    nc.gpsimd.memset(_epst.ap(), EPSP)
    nc.const_aps.aps[(F32, EPSP)] = _epst.ap()
    nc.all_engine_barrier()

    with TileContext(nc) as tc:
        with (
            tc.tile_pool(name="const", bufs=1) as cpool,
            tc.tile_pool(name="io", bufs=2) as io,
            tc.tile_pool(name="work", bufs=1) as work,
            tc.tile_pool(name="keep", bufs=1) as keep,
            tc.tile_pool(name="outp", bufs=3) as outp,
            tc.tile_pool(name="psum", bufs=2, space="PSUM") as pps,
        ):
            from concourse.masks import make_identity
            ident = cpool.tile([128, 128], F32, name="ident")
            make_identity(nc, ident[:])
            identb = cpool.tile([128, 128], BF16, name="identb")
            A.copy(identb[:], ident[:])

            def body():
                dfx_arrs = {
                    b: _Arr(keep.tile([128, nx], BF16, tag=f"dfx{b}",
                                      name=f"dfx{b}"), 0)
                    for b in range(yb)
                }

                def x_chunk(c, pfx):
                    # both row-blocks in one call: tiles [128, yb, w]
                    F0 = c * xc - 1
                    WF = xc + 2
                    bH = c * xc - 3
                    hx = io.tile([128, yb, xc + 6], BF16, tag="hx",
                                 name="hx")
                    nc.sync.dma_start(
                        out=hx[:],
                        in_=hx_d[:, :, c * xc:c * xc + xc + 6],
                    )
                    up = io.tile([128, yb, xc + 1], BF16, tag="upx",
                                 name="upx")
                    nc.sync.dma_start(
                        out=up[:],
                        in_=upx_d[:, :, c * xc:c * xc + xc + 1],
                    )
                    um = io.tile([128, yb, xc + 1], BF16, tag="umx",
                                 name="umx")
                    nc.sync.dma_start(
                        out=um[:],
                        in_=umx_d[:, :, c * xc:c * xc + xc + 1],
                    )
                    HB = _Arr(hx, bH)
                    UP = _Arr(up, F0)
                    UM = _Arr(um, F0)
                    sl = _Slots(work, [128, yb, 0], nslots=nslots,
                                bufs=work_bufs, prefix=pfx)
                    fe = yield from _flux_v2(nc, sl, HB, UP, UM, F0, WF,
                                             xc + 1, asg)
                    # dfx = fe'[f] - fe'[f-1]  (already -flux/DX scaled)
                    o = c * xc - fe.base
                    for b in range(yb):
                        V.tensor_tensor(
                            dfx_arrs[b].tile[:, c * xc:(c + 1) * xc],
                            fe.tile[:, b, o:o + xc],
                            fe.tile[:, b, o - 1:o - 1 + xc], SUB)
                    sl.free("fe")
                    yield

                def y_group(g, pfx):
                    ht = io.tile([128, g_seg, YW], BF16, tag="hy", name="hy")
                    nc.sync.dma_start(
                        out=ht[:], in_=hty_d[g * 128:(g + 1) * 128, :, :])
                    upt = io.tile([128, g_seg, VW], BF16, tag="upy",
                                  name="upy")
                    nc.sync.dma_start(
                        out=upt[:], in_=upy_d[g * 128:(g + 1) * 128, :, :])
                    umt = io.tile([128, g_seg, VW], BF16, tag="umy",
                                  name="umy")
                    nc.sync.dma_start(
                        out=umt[:], in_=umy_d[g * 128:(g + 1) * 128, :, :])
                    Hy = _Arr(ht, -3)
                    UPy = _Arr(upt, -1)
                    UMy = _Arr(umt, -1)
                    sl = _Slots(work, [128, g_seg, 0], nslots=nslots,
                                bufs=work_bufs, prefix=pfx)
                    fn = yield from _flux_v2(nc, sl, Hy, UPy, UMy, -1,
                                             rpc + 2, rpc + 1, asg)
                    dfy = work.tile([128, g_seg, rpc], BF16,
                                    tag=f"dfy{pfx}", name="dfy",
                                    bufs=work_bufs)
                    V.tensor_tensor(dfy[:], fn.win(0, rpc),
                                    fn.win(-1, rpc), SUB)
                    sl.free("fe")
                    yield

                    mcw = 128 * g_seg
                    for b in range(yb):
                        zyp = pps.tile([128, mcw], BF16, tag="zyp",
                                       name="zyp")
                        for k in range(g_seg):
                            nc.tensor.transpose(
                                zyp[:, k * 128:(k + 1) * 128],
                                dfy[:, k, b * 128:b * 128 + 128],
                                identb[:],
                            )
                        ot = outp.tile([128, mcw], F32, tag="outsb",
                                       name="outsb")
                        V.tensor_tensor(
                            ot[:],
                            dfx_arrs[b].tile[:, g * mcw:(g + 1) * mcw],
                            zyp[:], ADD)
                        nc.sync.dma_start(
                            out=out_d[b * 128:b * 128 + 128,
                                      g * mcw:(g + 1) * mcw],
                            in_=ot[:],
                        )
                        yield

                gens = []
                i = 0
                for c in range(nchunk):
                    gens.append(x_chunk(c, "ABC"[i % npfx]))
                    i += 1
                for g in range(ng):
                    gens.append(y_group(g, "ABC"[i % npfx]))
                    i += 1
                _drive(gens, prime=prime, depth=depth)

            if hw_loop:
                with tc.For_i(0, reps):
                    body()
            else:
                for _ in range(reps):
                    body()
    nc.compile()
    return nc


# --------------------------------------------------------------------------
# Host side
# --------------------------------------------------------------------------
def _bf16(x):
    import ml_dtypes
    return np.asarray(x, dtype=ml_dtypes.bfloat16)


def make_shards(h, u, v, n_cores=N_CORES):
    rpc = h.shape[0] // n_cores
    ny, nx = h.shape
    ng = nx // (128 * G)
    hp = np.pad(h, ((3, 3), (3, 3)), mode="edge")      # [ny+6, nx+6]
    hxb = _bf16(hp[3:3 + ny, :])                       # [ny, nx+6]
    uf = np.concatenate([u[:, :1], u], axis=1)         # col j = u[:, j-1]
    upx = _bf16(np.maximum(uf, 0.0) * (-1.0 / DX))
    umx = _bf16(np.minimum(uf, 0.0) * (-1.0 / DX))
    hty = _bf16(np.ascontiguousarray(hp[:, 3:3 + nx].T))   # [nx, ny+6]
    vf = np.concatenate([v[:1, :], v], axis=0)         # row j = v[j-1]
    upy = _bf16(np.ascontiguousarray(
        (np.maximum(vf, 0.0) * (-1.0 / DY)).T))        # [nx, ny+1]
    umy = _bf16(np.ascontiguousarray(
        (np.minimum(vf, 0.0) * (-1.0 / DY)).T))

    def ytile(a, r0, w):
        # [nx, w] -> [ng*128, G, w] with [g*128+p, k, :] = a[(g*G+k)*128+p]
        s = np.ascontiguousarray(a[:, r0:r0 + w])
        s = s.reshape(ng, G, 128, w).transpose(0, 2, 1, 3)
        return np.ascontiguousarray(s.reshape(ng * 128, G, w))

    def xtile(a, r0):
        # [rpc, w] -> [128, yb, w] with [p, b, :] = a[r0 + b*128 + p]
        yb = rpc // 128
        s = np.ascontiguousarray(a[r0:r0 + rpc, :])
        return np.ascontiguousarray(
            s.reshape(yb, 128, a.shape[1]).transpose(1, 0, 2))

    maps = []
    for i in range(n_cores):
        r0 = i * rpc
        maps.append({
            "hx": xtile(hxb, r0),
            "upx": xtile(upx, r0),
            "umx": xtile(umx, r0),
            "hty": ytile(hty, r0, rpc + 6),
            "upy": ytile(upy, r0, rpc + 1),
            "umy": ytile(umy, r0, rpc + 1),
        })
    return maps


_NC_CACHE = {}


def kernel(h, u, v):
    h = np.asarray(h, dtype=np.float32)
    u = np.asarray(u, dtype=np.float32)
    v = np.asarray(v, dtype=np.float32)
    assert h.shape == (NY, NX), h.shape

    from concourse.bass_utils import run_bass_kernel_spmd

    if "main" not in _NC_CACHE:
        _NC_CACHE["main"] = build_program()
    nc = _NC_CACHE["main"]

    in_maps = make_shards(h, u, v)
    res = run_bass_kernel_spmd(nc, in_maps, list(range(N_CORES)))
    out = np.concatenate([res.results[i]["out"] for i in range(N_CORES)],
                         axis=0)
    out[:2, :] = 0.0
    out[-2:, :] = 0.0
    out[:, :2] = 0.0
    out[:, -2:] = 0.0
    return out
